# revision 26
# baseline (speedup 1.0000x reference)
"""Trainium2 Bass kernel for nn_CaptionModel (GRU caption decoder).

Model: h0 = feat; x0 = embed[<SOS>]; 200 GRU steps where the output hidden
state is fed back as the next input (x_t = h_t for t >= 1), then a linear
projection of every hidden state to vocab logits, output [B, V, T].

Since x_t == h_t for t >= 1 the two GRU matmuls fuse into one 2048-wide
gate matmul G = h @ Wc.T + bc with Wc = [w_ih_r+w_hh_r; w_ih_z+w_hh_z;
w_hh_n; w_ih_n], gates r = sig(G0), z = sig(G1), n = tanh(G3 + r*G2),
h' = (1-z)*n + z*h.

Layout: GATE-MAJOR, fp16 matmuls. Each core holds batch BD=32. The PE
computes G^T [2048 gates -> 16 chunks of 128 partitions, 32 batch free]
with the 128x128 weight blocks as stationary operands and the (tiny)
hidden state as the moving operand: 16 bias rows + 128 weight matmuls of
32 moving rows each (~0.6us PE busy vs ~3.5us for weight-moving layouts;
stationary loads are pipelined). h' = u + e is never materialized on the
critical path: u = (1-z)*h (ready early) and e = (1-z)*n are fed to the
PE as TWO accumulating moving operands, removing the final join from the
recurrence cycle. z1m = 1-z comes directly from sigmoid(-zpre)
(scale=-1), and the u path (t1 = z1m*h, u = h - t1) runs off-chain on
Pool/DVE. The vocab projection is interleaved every 4 steps (lagged) to
fill PE idle time; hidden history is kept gate-major fp16 and projected
with the same stationary-weight trick.

Sharding: pure data parallelism, batch 256 -> 32 per core on 8 cores,
weights replicated.
"""

import os
from contextlib import ExitStack

import numpy as np

import concourse.bass as bass
import concourse.tile as tile
from concourse import bacc, mybir
from concourse.bass_utils import run_bass_kernel_spmd

B, H, VOCAB = 256, 512, 100
STEPS = int(os.environ.get("KERNEL_STEPS", "200"))
NCORES = 8
BD = B // NCORES  # 32
F16 = mybir.dt.float16
F32 = mybir.dt.float32
SIG = mybir.ActivationFunctionType.Sigmoid
TANH = mybir.ActivationFunctionType.Tanh

# gate order in both the weight blocks and the PSUM column regions
# r [0:128] z [128:256] hn [256:384] in [384:512]
GATES = ("r", "z", "hn", "in")
GI = {g: i for i, g in enumerate(GATES)}
PROJ_EVERY = 4


def _blk(g, q, c):
    return ((GI[g] * 4 + q) * 4 + c) * 128


def _build(steps: int):
    nc = bacc.Bacc("TRN2", target_bir_lowering=False, debug=False,
                   num_devices=NCORES)
    T1 = steps + 1

    wst_d = nc.dram_tensor("wst", [128, 64 * 128], F16, kind="ExternalInput").ap()
    wst0_d = nc.dram_tensor("wst0", [128, 32 * 128], F16, kind="ExternalInput").ap()
    h0_d = nc.dram_tensor("h0", [128, 128], F16, kind="ExternalInput").ap()
    bt_d = nc.dram_tensor("bt", [1, 2048], F16, kind="ExternalInput").ap()
    bt0_d = nc.dram_tensor("bt0", [1, 2048], F16, kind="ExternalInput").ap()
    ones_d = nc.dram_tensor("ones", [1, BD], F16, kind="ExternalInput").ap()
    pjt_d = nc.dram_tensor("pjt", [128, 4 * VOCAB], F16, kind="ExternalInput").ap()
    pjb_d = nc.dram_tensor("pjb", [VOCAB, 1], F32, kind="ExternalInput").ap()
    out_d = nc.dram_tensor("out", [BD, VOCAB, steps], F32,
                           kind="ExternalOutput").ap()

    with tile.TileContext(nc) as tc, ExitStack() as ctx:
        sg = ctx.enter_context(tc.tile_pool(name="sg", bufs=1))
        wk = ctx.enter_context(tc.tile_pool(name="wk", bufs=2))

        wst = sg.tile([128, 64 * 128], F16)
        nc.sync.dma_start(out=wst, in_=wst_d)
        wst0 = sg.tile([128, 32 * 128], F16)
        nc.sync.dma_start(out=wst0, in_=wst0_d)
        bt = sg.tile([1, 2048], F16)
        nc.sync.dma_start(out=bt, in_=bt_d)
        bt0 = sg.tile([1, 2048], F16)
        nc.sync.dma_start(out=bt0, in_=bt0_d)
        ones = sg.tile([1, BD], F16)
        nc.sync.dma_start(out=ones, in_=ones_d)
        pjt = sg.tile([128, 4 * VOCAB], F16)
        nc.sync.dma_start(out=pjt, in_=pjt_d)
        pjb = sg.tile([VOCAB, 1], F32)
        nc.sync.dma_start(out=pjb, in_=pjb_d)
        hist = sg.tile([128, 4, T1, BD], F16, name="hist")
        nc.sync.dma_start(out=hist[:, :, 0, :],
                          in_=h0_d.rearrange("p (q b) -> p q b", q=4))
        stage = sg.tile([VOCAB, BD * steps], F32, name="stage")

        with tc.tile_pool(name="gps", bufs=1, space="PSUM") as gpool, \
             tc.tile_pool(name="pps", bufs=2, space="PSUM") as ppool:
            G = [gpool.tile([128, 512], F32, tag=f"G{i}", name=f"G{i}")
                 for i in range(3)]

            # One PSUM accumulation group per G bank per step: start=True
            # zeroes the WHOLE 2KB zero region, so only the very first
            # matmul into the bank may carry start, and only the very last
            # carries stop.
            def bias_mms(t, bias):
                g = G[t % 3]
                first = True
                for gate in GATES:
                    for q in range(4):
                        nc.tensor.matmul(
                            g[:, GI[gate] * 128 + q * BD:
                              GI[gate] * 128 + (q + 1) * BD],
                            bias[:, GI[gate] * 512 + q * 128:
                                 GI[gate] * 512 + (q + 1) * 128],
                            ones, start=first, stop=False,
                            skip_group_check=True)
                        first = False

            def w_mms(t, w, rhs4, last, skip=(), cs=(0, 1, 2, 3)):
                g = G[t % 3]
                emitted = []
                for gate in GATES:
                    if gate in skip:
                        continue
                    for q in range(4):
                        for c in cs:
                            emitted.append((gate, q, c))
                for i, (gate, q, c) in enumerate(emitted):
                    col = GI[gate] * 128 + q * BD
                    if w is wst0 and gate in ("r", "z"):
                        wt, base = wst0, ((GI[gate] * 4 + q) * 4 + c) * 128
                    else:
                        wt, base = wst, _blk(gate, q, c)
                    nc.tensor.matmul(
                        g[:, col:col + BD],
                        wt[:, base:base + 128],
                        rhs4[c], start=False,
                        stop=(last and i == len(emitted) - 1),
                        skip_group_check=True)

            def proj_rows(r0, nrows):
                # P free dim iterates (b, t) so the stage (b-major,
                # t-minor) write and the final DMA stay contiguous in t
                Pfull = ppool.tile([VOCAB, 512], F32, tag="P", name="Pfull")
                P = Pfull[:, 0:BD * nrows]
                for c in range(4):
                    rhs = hist[:, c, r0:r0 + nrows, :].rearrange(
                        "p t b -> p b t")
                    nc.tensor.matmul(P, pjt[:, c * VOCAB:(c + 1) * VOCAB], rhs,
                                     start=(c == 0), stop=(c == 3))
                st_sl = stage.rearrange("p (b t) -> p b t", b=BD)[
                    :, :, r0 - 1:r0 - 1 + nrows]
                nc.vector.tensor_scalar_add(
                    st_sl, P.rearrange("p (b t) -> p b t", b=BD), pjb)

            # t=0: bias0 + w_hh matmuls on h0 (in-region is bias-only)
            bias_mms(0, bt0)
            w_mms(0, wst0, [hist[:, c, 0, :] for c in range(4)], last=True,
                  skip=("in",))

            # bias mms for step t+1 are emitted between step t's u and e
            # matmul batches: they execute inside the previous burst's
            # shadow instead of lengthening the critical burst
            if steps > 1:
                bias_mms(1, bt)
            next_proj = 1
            for t in range(steps):
                g = G[t % 3]
                rz_s = wk.tile([128, 256], F32, tag="rz")
                a_s = wk.tile([128, 128], F32, tag="a")
                b_s = wk.tile([128, 128], F32, tag="b")
                n16 = wk.tile([128, 128], F16, tag="n")
                t1 = wk.tile([128, 4, BD], F16, tag="t1")
                u16 = wk.tile([128, 4, BD], F16, tag="u")
                e16 = wk.tile([128, 4, BD], F16, tag="e")

                # critical cycle: z1m -> t1 -> u -> u-matmuls -> e-matmuls.
                # z1m = sigmoid(z psum) = 1-z (z weights negated host-side)
                # goes FIRST on Act: its update feeds Pool (u path) without
                # coalescing into r's Act->DVE update.
                z1mf = rz_s[:, 128:256]
                z1mf4 = z1mf.rearrange("p (q b) -> p q b", q=4)
                r_s = rz_s[:, 0:128]
                nc.scalar.activation(rz_s, g[:, 0:256], SIG)
                # u path: t1 = z1m*h kept in f32 so u = h - t1 rounds to
                # fp16 only once, relative to u's own magnitude.  First half
                # on Pool (fires right after z1m), second half on DVE after
                # the a/b chain.
                nc.gpsimd.tensor_mul(t1[:, 0:2, :], z1mf4[:, 0:2, :],
                                     hist[:, 0:2, t, :])
                nc.gpsimd.tensor_sub(u16[:, 0:2, :], hist[:, 0:2, t, :],
                                     t1[:, 0:2, :])
                # a/b/n/e in q-halves: the first half reaches the PE while
                # the second is still in flight
                for h0, h1 in ((0, 64), (64, 128)):
                    nc.vector.tensor_mul(a_s[:, h0:h1], r_s[:, h0:h1],
                                         g[:, 256 + h0:256 + h1])
                    nc.vector.tensor_add(b_s[:, h0:h1], a_s[:, h0:h1],
                                         g[:, 384 + h0:384 + h1])
                nc.vector.tensor_mul(t1[:, 2:4, :], z1mf4[:, 2:4, :],
                                     hist[:, 2:4, t, :])
                nc.vector.tensor_sub(u16[:, 2:4, :], hist[:, 2:4, t, :],
                                     t1[:, 2:4, :])
                n4 = n16.rearrange("p (q b) -> p q b", q=4)
                nc.scalar.activation(n16[:, 0:64], b_s[:, 0:64], TANH)
                nc.scalar.activation(n16[:, 64:128], b_s[:, 64:128], TANH)
                nc.vector.tensor_mul(e16[:, 0:2, :], z1mf4[:, 0:2, :],
                                     n4[:, 0:2, :])
                nc.vector.tensor_mul(e16[:, 2:4, :], z1mf4[:, 2:4, :],
                                     n4[:, 2:4, :])
                # h join (off the recurrence cycle; feeds hist/proj/u-path)
                nc.gpsimd.tensor_add(hist[:, :, t + 1, :], u16, e16)

                if t + 1 < steps:
                    u4 = [u16[:, c, :] for c in range(4)]
                    e4 = [e16[:, c, :] for c in range(4)]
                    w_mms(t + 1, wst, u4, last=False, cs=(0, 1))
                    w_mms(t + 1, wst, u4, last=False, cs=(2, 3))
                    if t + 2 < steps:
                        bias_mms(t + 2, bt)
                    w_mms(t + 1, wst, e4, last=False, cs=(0, 1))
                    w_mms(t + 1, wst, e4, last=True, cs=(2, 3))
                    if next_proj + PROJ_EVERY <= t:
                        proj_rows(next_proj, PROJ_EVERY)
                        next_proj += PROJ_EVERY

            while next_proj <= steps:
                nrows = min(PROJ_EVERY, steps + 1 - next_proj)
                proj_rows(next_proj, nrows)
                next_proj += nrows

        nc.sync.dma_start(
            out=out_d.rearrange("b v t -> v b t"),
            in_=stage.rearrange("p (b t) -> p b t", b=BD))
    nc.compile()
    return nc


_CACHE = {}


def _get_nc(steps: int):
    if steps not in _CACHE:
        _CACHE[steps] = _build(steps)
    return _CACHE[steps]


def _prep_inputs(feat, embed_table, w_ih, w_hh, b_ih, b_hh, proj_w, proj_b):
    f32 = np.float32
    f16 = np.float16
    w_ih = np.asarray(w_ih, f32)
    w_hh = np.asarray(w_hh, f32)
    b_ih = np.asarray(b_ih, f32)
    b_hh = np.asarray(b_hh, f32)
    # fused gate weights, gate-major order r, z, hn, in
    # z gate negated: sigmoid(z psum) then directly equals 1 - z
    Wc = np.concatenate([w_ih[:H] + w_hh[:H],
                         -(w_ih[H:2 * H] + w_hh[H:2 * H]),
                         w_hh[2 * H:],
                         w_ih[2 * H:]], 0)          # [4H, H]
    bc = np.concatenate([b_ih[:H] + b_hh[:H],
                         -(b_ih[H:2 * H] + b_hh[H:2 * H]),
                         b_hh[2 * H:],
                         b_ih[2 * H:]], 0)          # [4H]

    x0 = np.asarray(embed_table, f32)[0]
    gi0 = w_ih @ x0 + b_ih                          # [3H]
    bc0 = np.concatenate([gi0[:H] + b_hh[:H],
                          -(gi0[H:2 * H] + b_hh[H:2 * H]),
                          b_hh[2 * H:],
                          gi0[2 * H:]], 0)          # [4H]
    W0 = np.concatenate([w_hh[:H], -w_hh[H:2 * H]], 0)  # [2H, H] r,z step-0

    # stationary blocks: wst[kp, ((g*4+q)*4+c)*128 + m] = Wc[g*512+q*128+m,
    #                                                        c*128+kp]
    wst = np.empty((128, 64 * 128), f32)
    for g in range(4):
        for q in range(4):
            for c in range(4):
                blk = ((g * 4 + q) * 4 + c) * 128
                wst[:, blk:blk + 128] = Wc[g * 512 + q * 128:
                                           g * 512 + (q + 1) * 128,
                                           c * 128:(c + 1) * 128].T
    wst0 = np.empty((128, 32 * 128), f32)
    for g in range(2):
        for q in range(4):
            for c in range(4):
                blk = ((g * 4 + q) * 4 + c) * 128
                wst0[:, blk:blk + 128] = W0[g * 512 + q * 128:
                                            g * 512 + (q + 1) * 128,
                                            c * 128:(c + 1) * 128].T

    proj_w = np.asarray(proj_w, f32)                # [V, H]
    pjt = np.empty((128, 4 * VOCAB), f32)
    for c in range(4):
        pjt[:, c * VOCAB:(c + 1) * VOCAB] = proj_w[:, c * 128:(c + 1) * 128].T

    feat = np.asarray(feat, f32)
    common = {
        "wst": wst.astype(f16),
        "wst0": wst0.astype(f16),
        "bt": bc.reshape(1, 2048).astype(f16),
        "bt0": bc0.reshape(1, 2048).astype(f16),
        "ones": np.ones((1, BD), f16),
        "pjt": pjt.astype(f16),
        "pjb": np.asarray(proj_b, f32).reshape(VOCAB, 1),
    }
    maps = []
    for i in range(NCORES):
        fs = feat[i * BD:(i + 1) * BD]              # [BD, H]
        h0g = np.ascontiguousarray(
            fs.T.reshape(4, 128, BD).transpose(1, 0, 2).reshape(128, 128))
        maps.append(dict(common, h0=h0g.astype(f16)))
    return maps


def kernel(feat, embed_table, w_ih, w_hh, b_ih, b_hh, proj_w, proj_b,
           _trace=False):
    nc = _get_nc(STEPS)
    in_maps = _prep_inputs(feat, embed_table, w_ih, w_hh, b_ih, b_hh,
                           proj_w, proj_b)
    res = run_bass_kernel_spmd(nc, in_maps, list(range(NCORES)), trace=_trace)
    out = np.concatenate([res.results[i]["out"] for i in range(NCORES)], 0)
    if _trace:
        kernel.last_exec_time_ns = res.exec_time_ns
        kernel.last_results = res
    return out


# revision 29
# speedup vs baseline: 1.0155x; 1.0155x over previous
"""Trainium2 Bass kernel for nn_CaptionModel (GRU caption decoder).

Model: h0 = feat; x0 = embed[<SOS>]; 200 GRU steps where the output hidden
state is fed back as the next input (x_t = h_t for t >= 1), then a linear
projection of every hidden state to vocab logits, output [B, V, T].

Since x_t == h_t for t >= 1 the two GRU matmuls fuse into one 2048-wide
gate matmul G = h @ Wc.T + bc with Wc = [w_ih_r+w_hh_r; w_ih_z+w_hh_z;
w_hh_n; w_ih_n], gates r = sig(G0), z = sig(G1), n = tanh(G3 + r*G2),
h' = (1-z)*n + z*h.

Layout: GATE-MAJOR, fp16 matmuls. Each core holds batch BD=32. The PE
computes G^T [2048 gates -> 16 chunks of 128 partitions, 32 batch free]
with the 128x128 weight blocks as stationary operands and the (tiny)
hidden state as the moving operand: 16 bias rows + 128 weight matmuls of
32 moving rows each (~0.6us PE busy vs ~3.5us for weight-moving layouts;
stationary loads are pipelined). h' = u + e is never materialized on the
critical path: u = (1-z)*h (ready early) and e = (1-z)*n are fed to the
PE as TWO accumulating moving operands, removing the final join from the
recurrence cycle. z1m = 1-z comes directly from sigmoid(-zpre)
(scale=-1), and the u path (t1 = z1m*h, u = h - t1) runs off-chain on
Pool/DVE. The vocab projection is interleaved every 4 steps (lagged) to
fill PE idle time; hidden history is kept gate-major fp16 and projected
with the same stationary-weight trick.

Sharding: pure data parallelism, batch 256 -> 32 per core on 8 cores,
weights replicated.
"""

import os
from contextlib import ExitStack

import numpy as np

import concourse.bass as bass
import concourse.tile as tile
from concourse import bacc, mybir
from concourse.bass_utils import run_bass_kernel_spmd

B, H, VOCAB = 256, 512, 100
STEPS = int(os.environ.get("KERNEL_STEPS", "200"))
NCORES = 8
BD = B // NCORES  # 32
F16 = mybir.dt.float16
F32 = mybir.dt.float32
SIG = mybir.ActivationFunctionType.Sigmoid
TANH = mybir.ActivationFunctionType.Tanh

# gate order in both the weight blocks and the PSUM column regions
# r [0:128] z [128:256] hn [256:384] in [384:512]
GATES = ("r", "z", "hn", "in")
GI = {g: i for i, g in enumerate(GATES)}
PROJ_EVERY = 4


def _blk(g, q, c):
    return ((GI[g] * 4 + q) * 4 + c) * 128


def _build(steps: int):
    nc = bacc.Bacc("TRN2", target_bir_lowering=False, debug=False,
                   num_devices=NCORES)
    T1 = steps + 1

    wst_d = nc.dram_tensor("wst", [128, 64 * 128], F16, kind="ExternalInput").ap()
    wst0_d = nc.dram_tensor("wst0", [128, 32 * 128], F16, kind="ExternalInput").ap()
    h0_d = nc.dram_tensor("h0", [128, 128], F16, kind="ExternalInput").ap()
    bt_d = nc.dram_tensor("bt", [1, 2048], F16, kind="ExternalInput").ap()
    bt0_d = nc.dram_tensor("bt0", [1, 2048], F16, kind="ExternalInput").ap()
    ones_d = nc.dram_tensor("ones", [1, BD], F16, kind="ExternalInput").ap()
    pjt_d = nc.dram_tensor("pjt", [128, 4 * VOCAB], F16, kind="ExternalInput").ap()
    pjb_d = nc.dram_tensor("pjb", [VOCAB, 1], F32, kind="ExternalInput").ap()
    out_d = nc.dram_tensor("out", [BD, VOCAB, steps], F32,
                           kind="ExternalOutput").ap()

    with tile.TileContext(nc) as tc, ExitStack() as ctx:
        sg = ctx.enter_context(tc.tile_pool(name="sg", bufs=1))
        wk = ctx.enter_context(tc.tile_pool(name="wk", bufs=2))

        wst = sg.tile([128, 64 * 128], F16)
        nc.sync.dma_start(out=wst, in_=wst_d)
        wst0 = sg.tile([128, 32 * 128], F16)
        nc.sync.dma_start(out=wst0, in_=wst0_d)
        bt = sg.tile([1, 2048], F16)
        nc.sync.dma_start(out=bt, in_=bt_d)
        bt0 = sg.tile([1, 2048], F16)
        nc.sync.dma_start(out=bt0, in_=bt0_d)
        ones = sg.tile([1, BD], F16)
        nc.sync.dma_start(out=ones, in_=ones_d)
        pjt = sg.tile([128, 4 * VOCAB], F16)
        nc.sync.dma_start(out=pjt, in_=pjt_d)
        pjb = sg.tile([VOCAB, 1], F32)
        nc.sync.dma_start(out=pjb, in_=pjb_d)
        hist = sg.tile([128, 4, T1, BD], F16, name="hist")
        nc.sync.dma_start(out=hist[:, :, 0, :],
                          in_=h0_d.rearrange("p (q b) -> p q b", q=4))
        stage = sg.tile([VOCAB, BD * steps], F32, name="stage")

        with tc.tile_pool(name="gps", bufs=1, space="PSUM") as gpool, \
             tc.tile_pool(name="pps", bufs=2, space="PSUM") as ppool:
            G = [gpool.tile([128, 512], F32, tag=f"G{i}", name=f"G{i}")
                 for i in range(3)]

            # One PSUM accumulation group per G bank per step: start=True
            # zeroes the WHOLE 2KB zero region, so only the very first
            # matmul into the bank may carry start, and only the very last
            # carries stop.
            def bias_mms(t, bias):
                g = G[t % 3]
                first = True
                for gate in GATES:
                    for q in range(4):
                        nc.tensor.matmul(
                            g[:, GI[gate] * 128 + q * BD:
                              GI[gate] * 128 + (q + 1) * BD],
                            bias[:, GI[gate] * 512 + q * 128:
                                 GI[gate] * 512 + (q + 1) * 128],
                            ones, start=first, stop=False,
                            skip_group_check=True)
                        first = False

            def w_mms(t, w, rhs4, last, skip=(), cs=(0, 1, 2, 3)):
                g = G[t % 3]
                emitted = []
                for gate in GATES:
                    if gate in skip:
                        continue
                    for q in range(4):
                        for c in cs:
                            emitted.append((gate, q, c))
                for i, (gate, q, c) in enumerate(emitted):
                    col = GI[gate] * 128 + q * BD
                    if w is wst0 and gate in ("r", "z"):
                        wt, base = wst0, ((GI[gate] * 4 + q) * 4 + c) * 128
                    else:
                        wt, base = wst, _blk(gate, q, c)
                    nc.tensor.matmul(
                        g[:, col:col + BD],
                        wt[:, base:base + 128],
                        rhs4[c], start=False,
                        stop=(last and i == len(emitted) - 1),
                        skip_group_check=True)

            def proj_rows(r0, nrows):
                # P free dim iterates (b, t) so the stage (b-major,
                # t-minor) write and the final DMA stay contiguous in t
                Pfull = ppool.tile([VOCAB, 512], F32, tag="P", name="Pfull")
                P = Pfull[:, 0:BD * nrows]
                for c in range(4):
                    rhs = hist[:, c, r0:r0 + nrows, :].rearrange(
                        "p t b -> p b t")
                    nc.tensor.matmul(P, pjt[:, c * VOCAB:(c + 1) * VOCAB], rhs,
                                     start=(c == 0), stop=(c == 3))
                st_sl = stage.rearrange("p (b t) -> p b t", b=BD)[
                    :, :, r0 - 1:r0 - 1 + nrows]
                nc.vector.tensor_scalar_add(
                    st_sl, P.rearrange("p (b t) -> p b t", b=BD), pjb)

            # t=0: bias0 + w_hh matmuls on h0 (in-region is bias-only)
            bias_mms(0, bt0)
            w_mms(0, wst0, [hist[:, c, 0, :] for c in range(4)], last=True,
                  skip=("in",))

            # bias mms for step t+1 are emitted between step t's u and e
            # matmul batches: they execute inside the previous burst's
            # shadow instead of lengthening the critical burst
            if steps > 1:
                bias_mms(1, bt)
            next_proj = 1
            for t in range(steps):
                g = G[t % 3]
                rz_s = wk.tile([128, 256], F32, tag="rz")
                a_s = wk.tile([128, 128], F32, tag="a")
                b_s = wk.tile([128, 128], F32, tag="b")
                n16 = wk.tile([128, 128], F16, tag="n")
                t1 = wk.tile([128, 4, BD], F16, tag="t1")
                u16 = wk.tile([128, 4, BD], F16, tag="u")
                e16 = wk.tile([128, 4, BD], F16, tag="e")

                # critical cycle: z1m -> t1 -> u -> u-matmuls -> e-matmuls.
                # z1m = sigmoid(z psum) = 1-z (z weights negated host-side)
                # goes FIRST on Act: its update feeds Pool (u path) without
                # coalescing into r's Act->DVE update.
                z1mf = rz_s[:, 128:256]
                z1mf4 = z1mf.rearrange("p (q b) -> p q b", q=4)
                r_s = rz_s[:, 0:128]
                nc.scalar.activation(rz_s, g[:, 0:256], SIG)
                # u path: t1 = z1m*h kept in f32 so u = h - t1 rounds to
                # fp16 only once, relative to u's own magnitude.  First half
                # on Pool (fires right after z1m), second half on DVE after
                # the a/b chain.
                nc.gpsimd.tensor_mul(t1[:, 0:2, :], z1mf4[:, 0:2, :],
                                     hist[:, 0:2, t, :])
                nc.gpsimd.tensor_sub(u16[:, 0:2, :], hist[:, 0:2, t, :],
                                     t1[:, 0:2, :])
                # a/b/n/e in q-halves: the first half reaches the PE while
                # the second is still in flight
                for h0, h1 in ((0, 64), (64, 128)):
                    nc.vector.tensor_mul(a_s[:, h0:h1], r_s[:, h0:h1],
                                         g[:, 256 + h0:256 + h1])
                    nc.vector.tensor_add(b_s[:, h0:h1], a_s[:, h0:h1],
                                         g[:, 384 + h0:384 + h1])
                nc.vector.tensor_mul(t1[:, 2:4, :], z1mf4[:, 2:4, :],
                                     hist[:, 2:4, t, :])
                nc.vector.tensor_sub(u16[:, 2:4, :], hist[:, 2:4, t, :],
                                     t1[:, 2:4, :])
                n4 = n16.rearrange("p (q b) -> p q b", q=4)
                for qq in range(4):
                    nc.scalar.activation(n16[:, qq * 32:(qq + 1) * 32],
                                         b_s[:, qq * 32:(qq + 1) * 32], TANH)
                    nc.vector.tensor_mul(e16[:, qq, :], z1mf4[:, qq, :],
                                         n4[:, qq, :])
                # h join (off the recurrence cycle; feeds hist/proj/u-path)
                nc.gpsimd.tensor_add(hist[:, :, t + 1, :], u16, e16)

                if t + 1 < steps:
                    u4 = [u16[:, c, :] for c in range(4)]
                    e4 = [e16[:, c, :] for c in range(4)]
                    w_mms(t + 1, wst, u4, last=False, cs=(0, 1))
                    w_mms(t + 1, wst, u4, last=False, cs=(2, 3))
                    if t + 2 < steps:
                        bias_mms(t + 2, bt)
                    w_mms(t + 1, wst, e4, last=False, cs=(0,))
                    w_mms(t + 1, wst, e4, last=False, cs=(1,))
                    w_mms(t + 1, wst, e4, last=False, cs=(2,))
                    w_mms(t + 1, wst, e4, last=True, cs=(3,))
                    if next_proj + PROJ_EVERY <= t:
                        proj_rows(next_proj, PROJ_EVERY)
                        next_proj += PROJ_EVERY

            while next_proj <= steps:
                nrows = min(PROJ_EVERY, steps + 1 - next_proj)
                proj_rows(next_proj, nrows)
                next_proj += nrows

        nc.sync.dma_start(
            out=out_d.rearrange("b v t -> v b t"),
            in_=stage.rearrange("p (b t) -> p b t", b=BD))
    nc.compile()
    return nc


_CACHE = {}


def _get_nc(steps: int):
    if steps not in _CACHE:
        _CACHE[steps] = _build(steps)
    return _CACHE[steps]


def _prep_inputs(feat, embed_table, w_ih, w_hh, b_ih, b_hh, proj_w, proj_b):
    f32 = np.float32
    f16 = np.float16
    w_ih = np.asarray(w_ih, f32)
    w_hh = np.asarray(w_hh, f32)
    b_ih = np.asarray(b_ih, f32)
    b_hh = np.asarray(b_hh, f32)
    # fused gate weights, gate-major order r, z, hn, in
    # z gate negated: sigmoid(z psum) then directly equals 1 - z
    Wc = np.concatenate([w_ih[:H] + w_hh[:H],
                         -(w_ih[H:2 * H] + w_hh[H:2 * H]),
                         w_hh[2 * H:],
                         w_ih[2 * H:]], 0)          # [4H, H]
    bc = np.concatenate([b_ih[:H] + b_hh[:H],
                         -(b_ih[H:2 * H] + b_hh[H:2 * H]),
                         b_hh[2 * H:],
                         b_ih[2 * H:]], 0)          # [4H]

    x0 = np.asarray(embed_table, f32)[0]
    gi0 = w_ih @ x0 + b_ih                          # [3H]
    bc0 = np.concatenate([gi0[:H] + b_hh[:H],
                          -(gi0[H:2 * H] + b_hh[H:2 * H]),
                          b_hh[2 * H:],
                          gi0[2 * H:]], 0)          # [4H]
    W0 = np.concatenate([w_hh[:H], -w_hh[H:2 * H]], 0)  # [2H, H] r,z step-0

    # stationary blocks: wst[kp, ((g*4+q)*4+c)*128 + m] = Wc[g*512+q*128+m,
    #                                                        c*128+kp]
    wst = np.empty((128, 64 * 128), f32)
    for g in range(4):
        for q in range(4):
            for c in range(4):
                blk = ((g * 4 + q) * 4 + c) * 128
                wst[:, blk:blk + 128] = Wc[g * 512 + q * 128:
                                           g * 512 + (q + 1) * 128,
                                           c * 128:(c + 1) * 128].T
    wst0 = np.empty((128, 32 * 128), f32)
    for g in range(2):
        for q in range(4):
            for c in range(4):
                blk = ((g * 4 + q) * 4 + c) * 128
                wst0[:, blk:blk + 128] = W0[g * 512 + q * 128:
                                            g * 512 + (q + 1) * 128,
                                            c * 128:(c + 1) * 128].T

    proj_w = np.asarray(proj_w, f32)                # [V, H]
    pjt = np.empty((128, 4 * VOCAB), f32)
    for c in range(4):
        pjt[:, c * VOCAB:(c + 1) * VOCAB] = proj_w[:, c * 128:(c + 1) * 128].T

    feat = np.asarray(feat, f32)
    common = {
        "wst": wst.astype(f16),
        "wst0": wst0.astype(f16),
        "bt": bc.reshape(1, 2048).astype(f16),
        "bt0": bc0.reshape(1, 2048).astype(f16),
        "ones": np.ones((1, BD), f16),
        "pjt": pjt.astype(f16),
        "pjb": np.asarray(proj_b, f32).reshape(VOCAB, 1),
    }
    maps = []
    for i in range(NCORES):
        fs = feat[i * BD:(i + 1) * BD]              # [BD, H]
        h0g = np.ascontiguousarray(
            fs.T.reshape(4, 128, BD).transpose(1, 0, 2).reshape(128, 128))
        maps.append(dict(common, h0=h0g.astype(f16)))
    return maps


def kernel(feat, embed_table, w_ih, w_hh, b_ih, b_hh, proj_w, proj_b,
           _trace=False):
    nc = _get_nc(STEPS)
    in_maps = _prep_inputs(feat, embed_table, w_ih, w_hh, b_ih, b_hh,
                           proj_w, proj_b)
    res = run_bass_kernel_spmd(nc, in_maps, list(range(NCORES)), trace=_trace)
    out = np.concatenate([res.results[i]["out"] for i in range(NCORES)], 0)
    if _trace:
        kernel.last_exec_time_ns = res.exec_time_ns
        kernel.last_results = res
    return out


# revision 32
# speedup vs baseline: 1.0206x; 1.0050x over previous
"""Trainium2 Bass kernel for nn_CaptionModel (GRU caption decoder).

Model: h0 = feat; x0 = embed[<SOS>]; 200 GRU steps where the output hidden
state is fed back as the next input (x_t = h_t for t >= 1), then a linear
projection of every hidden state to vocab logits, output [B, V, T].

Since x_t == h_t for t >= 1 the two GRU matmuls fuse into one 2048-wide
gate matmul G = h @ Wc.T + bc with Wc = [w_ih_r+w_hh_r; w_ih_z+w_hh_z;
w_hh_n; w_ih_n], gates r = sig(G0), z = sig(G1), n = tanh(G3 + r*G2),
h' = (1-z)*n + z*h.

Layout: GATE-MAJOR, fp16 matmuls. Each core holds batch BD=32. The PE
computes G^T [2048 gates -> 16 chunks of 128 partitions, 32 batch free]
with the 128x128 weight blocks as stationary operands and the (tiny)
hidden state as the moving operand: 16 bias rows + 128 weight matmuls of
32 moving rows each (~0.6us PE busy vs ~3.5us for weight-moving layouts;
stationary loads are pipelined). h' = u + e is never materialized on the
critical path: u = (1-z)*h (ready early) and e = (1-z)*n are fed to the
PE as TWO accumulating moving operands, removing the final join from the
recurrence cycle. z1m = 1-z comes directly from sigmoid(-zpre)
(scale=-1), and the u path (t1 = z1m*h, u = h - t1) runs off-chain on
Pool/DVE. The vocab projection is interleaved every 4 steps (lagged) to
fill PE idle time; hidden history is kept gate-major fp16 and projected
with the same stationary-weight trick.

Sharding: pure data parallelism, batch 256 -> 32 per core on 8 cores,
weights replicated.
"""

import os
from contextlib import ExitStack

import numpy as np

import concourse.bass as bass
import concourse.tile as tile
from concourse import bacc, mybir
from concourse.bass_utils import run_bass_kernel_spmd

B, H, VOCAB = 256, 512, 100
STEPS = int(os.environ.get("KERNEL_STEPS", "200"))
NCORES = 8
BD = B // NCORES  # 32
F16 = mybir.dt.float16
F32 = mybir.dt.float32
SIG = mybir.ActivationFunctionType.Sigmoid
TANH = mybir.ActivationFunctionType.Tanh

# gate order in both the weight blocks and the PSUM column regions
# r [0:128] z [128:256] hn [256:384] in [384:512]
GATES = ("r", "z", "hn", "in")
GI = {g: i for i, g in enumerate(GATES)}
PROJ_EVERY = 4


def _blk(g, q, c):
    return ((GI[g] * 4 + q) * 4 + c) * 128


def _build(steps: int):
    nc = bacc.Bacc("TRN2", target_bir_lowering=False, debug=False,
                   num_devices=NCORES)
    T1 = steps + 1

    wst_d = nc.dram_tensor("wst", [128, 64 * 128], F16, kind="ExternalInput").ap()
    wst0_d = nc.dram_tensor("wst0", [128, 32 * 128], F16, kind="ExternalInput").ap()
    h0_d = nc.dram_tensor("h0", [128, 128], F16, kind="ExternalInput").ap()
    bt_d = nc.dram_tensor("bt", [1, 2048], F16, kind="ExternalInput").ap()
    bt0_d = nc.dram_tensor("bt0", [1, 2048], F16, kind="ExternalInput").ap()
    ones_d = nc.dram_tensor("ones", [1, BD], F16, kind="ExternalInput").ap()
    pjt_d = nc.dram_tensor("pjt", [128, 4 * VOCAB], F16, kind="ExternalInput").ap()
    pjb_d = nc.dram_tensor("pjb", [VOCAB, 1], F32, kind="ExternalInput").ap()
    out_d = nc.dram_tensor("out", [BD, VOCAB, steps], F32,
                           kind="ExternalOutput").ap()

    with tile.TileContext(nc) as tc, ExitStack() as ctx:
        sg = ctx.enter_context(tc.tile_pool(name="sg", bufs=1))
        wk = ctx.enter_context(tc.tile_pool(name="wk", bufs=2))

        wst = sg.tile([128, 64 * 128], F16)
        nc.sync.dma_start(out=wst, in_=wst_d)
        wst0 = sg.tile([128, 32 * 128], F16)
        nc.sync.dma_start(out=wst0, in_=wst0_d)
        bt = sg.tile([1, 2048], F16)
        nc.sync.dma_start(out=bt, in_=bt_d)
        bt0 = sg.tile([1, 2048], F16)
        nc.sync.dma_start(out=bt0, in_=bt0_d)
        ones = sg.tile([1, BD], F16)
        nc.sync.dma_start(out=ones, in_=ones_d)
        pjt = sg.tile([128, 4 * VOCAB], F16)
        nc.sync.dma_start(out=pjt, in_=pjt_d)
        pjb = sg.tile([VOCAB, 1], F32)
        nc.sync.dma_start(out=pjb, in_=pjb_d)
        hist = sg.tile([128, 4, T1, BD], F16, name="hist")
        nc.sync.dma_start(out=hist[:, :, 0, :],
                          in_=h0_d.rearrange("p (q b) -> p q b", q=4))
        stage = sg.tile([VOCAB, BD * steps], F32, name="stage")

        with tc.tile_pool(name="gps", bufs=1, space="PSUM") as gpool, \
             tc.tile_pool(name="pps", bufs=2, space="PSUM") as ppool:
            G = [gpool.tile([128, 512], F32, tag=f"G{i}", name=f"G{i}")
                 for i in range(3)]

            # One PSUM accumulation group per G bank per step: start=True
            # zeroes the WHOLE 2KB zero region, so only the very first
            # matmul into the bank may carry start, and only the very last
            # carries stop.
            def bias_mms(t, bias):
                g = G[t % 3]
                first = True
                for gate in GATES:
                    for q in range(4):
                        nc.tensor.matmul(
                            g[:, GI[gate] * 128 + q * BD:
                              GI[gate] * 128 + (q + 1) * BD],
                            bias[:, GI[gate] * 512 + q * 128:
                                 GI[gate] * 512 + (q + 1) * 128],
                            ones, start=first, stop=False,
                            skip_group_check=True)
                        first = False

            def w_mms(t, w, rhs4, last, skip=(), cs=(0, 1, 2, 3)):
                g = G[t % 3]
                emitted = []
                for gate in GATES:
                    if gate in skip:
                        continue
                    for q in range(4):
                        for c in cs:
                            emitted.append((gate, q, c))
                for i, (gate, q, c) in enumerate(emitted):
                    col = GI[gate] * 128 + q * BD
                    if w is wst0 and gate in ("r", "z"):
                        wt, base = wst0, ((GI[gate] * 4 + q) * 4 + c) * 128
                    else:
                        wt, base = wst, _blk(gate, q, c)
                    nc.tensor.matmul(
                        g[:, col:col + BD],
                        wt[:, base:base + 128],
                        rhs4[c], start=False,
                        stop=(last and i == len(emitted) - 1),
                        skip_group_check=True)

            def proj_rows(r0, nrows):
                # P free dim iterates (b, t) so the stage (b-major,
                # t-minor) write and the final DMA stay contiguous in t
                Pfull = ppool.tile([VOCAB, 512], F32, tag="P", name="Pfull")
                P = Pfull[:, 0:BD * nrows]
                for c in range(4):
                    rhs = hist[:, c, r0:r0 + nrows, :].rearrange(
                        "p t b -> p b t")
                    nc.tensor.matmul(P, pjt[:, c * VOCAB:(c + 1) * VOCAB], rhs,
                                     start=(c == 0), stop=(c == 3))
                st_sl = stage.rearrange("p (b t) -> p b t", b=BD)[
                    :, :, r0 - 1:r0 - 1 + nrows]
                nc.vector.tensor_scalar_add(
                    st_sl, P.rearrange("p (b t) -> p b t", b=BD), pjb)

            # t=0: bias0 + w_hh matmuls on h0 (in-region is bias-only)
            bias_mms(0, bt0)
            w_mms(0, wst0, [hist[:, c, 0, :] for c in range(4)], last=True,
                  skip=("in",))

            # bias mms for step t+1 are emitted between step t's u and e
            # matmul batches: they execute inside the previous burst's
            # shadow instead of lengthening the critical burst
            if steps > 1:
                bias_mms(1, bt)
            next_proj = 1
            dma_done = 0
            DMA_CHUNK = 50
            for t in range(steps):
                g = G[t % 3]
                rz_s = wk.tile([128, 256], F32, tag="rz")
                a_s = wk.tile([128, 128], F32, tag="a")
                b_s = wk.tile([128, 128], F32, tag="b")
                n16 = wk.tile([128, 128], F16, tag="n")
                t1 = wk.tile([128, 4, BD], F16, tag="t1")
                u16 = wk.tile([128, 4, BD], F16, tag="u")
                e16 = wk.tile([128, 4, BD], F16, tag="e")

                # critical cycle: z1m -> t1 -> u -> u-matmuls -> e-matmuls.
                # z1m = sigmoid(z psum) = 1-z (z weights negated host-side)
                # goes FIRST on Act: its update feeds Pool (u path) without
                # coalescing into r's Act->DVE update.
                z1mf = rz_s[:, 128:256]
                z1mf4 = z1mf.rearrange("p (q b) -> p q b", q=4)
                r_s = rz_s[:, 0:128]
                nc.scalar.activation(rz_s, g[:, 0:256], SIG)
                # u path: t1 = z1m*h kept in f32 so u = h - t1 rounds to
                # fp16 only once, relative to u's own magnitude.  First half
                # on Pool (fires right after z1m), second half on DVE after
                # the a/b chain.
                nc.gpsimd.tensor_mul(t1[:, 0:2, :], z1mf4[:, 0:2, :],
                                     hist[:, 0:2, t, :])
                nc.gpsimd.tensor_sub(u16[:, 0:2, :], hist[:, 0:2, t, :],
                                     t1[:, 0:2, :])
                # a/b/n/e in q-halves: the first half reaches the PE while
                # the second is still in flight
                for h0, h1 in ((0, 64), (64, 128)):
                    nc.vector.tensor_mul(a_s[:, h0:h1], r_s[:, h0:h1],
                                         g[:, 256 + h0:256 + h1])
                    nc.vector.tensor_add(b_s[:, h0:h1], a_s[:, h0:h1],
                                         g[:, 384 + h0:384 + h1])
                nc.vector.tensor_mul(t1[:, 2:4, :], z1mf4[:, 2:4, :],
                                     hist[:, 2:4, t, :])
                nc.vector.tensor_sub(u16[:, 2:4, :], hist[:, 2:4, t, :],
                                     t1[:, 2:4, :])
                n4 = n16.rearrange("p (q b) -> p q b", q=4)
                for qq in range(4):
                    nc.scalar.activation(n16[:, qq * 32:(qq + 1) * 32],
                                         b_s[:, qq * 32:(qq + 1) * 32], TANH)
                    nc.vector.tensor_mul(e16[:, qq, :], z1mf4[:, qq, :],
                                         n4[:, qq, :])
                # h join (off the recurrence cycle; feeds hist/proj/u-path)
                nc.gpsimd.tensor_add(hist[:, :, t + 1, :], u16, e16)

                if t + 1 < steps:
                    u4 = [u16[:, c, :] for c in range(4)]
                    e4 = [e16[:, c, :] for c in range(4)]
                    w_mms(t + 1, wst, u4, last=False, cs=(0, 1))
                    w_mms(t + 1, wst, u4, last=False, cs=(2, 3))
                    if t + 2 < steps:
                        bias_mms(t + 2, bt)
                    w_mms(t + 1, wst, e4, last=False, cs=(0,))
                    w_mms(t + 1, wst, e4, last=False, cs=(1,))
                    w_mms(t + 1, wst, e4, last=False, cs=(2,))
                    w_mms(t + 1, wst, e4, last=True, cs=(3,))
                    if next_proj + PROJ_EVERY <= t:
                        proj_rows(next_proj, PROJ_EVERY)
                        next_proj += PROJ_EVERY
                        # stream finalized 50-row output chunks during the
                        # recurrence instead of one big DMA at the end
                        while dma_done + DMA_CHUNK < next_proj:
                            lo = dma_done
                            nc.sync.dma_start(
                                out=out_d[:, :, lo:lo + DMA_CHUNK].rearrange(
                                    "b v t -> v b t"),
                                in_=stage.rearrange("p (b t) -> p b t", b=BD)[
                                    :, :, lo:lo + DMA_CHUNK])
                            dma_done += DMA_CHUNK

            while next_proj <= steps:
                nrows = min(PROJ_EVERY, steps + 1 - next_proj)
                proj_rows(next_proj, nrows)
                next_proj += nrows

        if dma_done < steps:
            nc.sync.dma_start(
                out=out_d[:, :, dma_done:].rearrange("b v t -> v b t"),
                in_=stage.rearrange("p (b t) -> p b t", b=BD)[:, :, dma_done:])
    nc.compile()
    return nc


_CACHE = {}


def _get_nc(steps: int):
    if steps not in _CACHE:
        _CACHE[steps] = _build(steps)
    return _CACHE[steps]


def _prep_inputs(feat, embed_table, w_ih, w_hh, b_ih, b_hh, proj_w, proj_b):
    f32 = np.float32
    f16 = np.float16
    w_ih = np.asarray(w_ih, f32)
    w_hh = np.asarray(w_hh, f32)
    b_ih = np.asarray(b_ih, f32)
    b_hh = np.asarray(b_hh, f32)
    # fused gate weights, gate-major order r, z, hn, in
    # z gate negated: sigmoid(z psum) then directly equals 1 - z
    Wc = np.concatenate([w_ih[:H] + w_hh[:H],
                         -(w_ih[H:2 * H] + w_hh[H:2 * H]),
                         w_hh[2 * H:],
                         w_ih[2 * H:]], 0)          # [4H, H]
    bc = np.concatenate([b_ih[:H] + b_hh[:H],
                         -(b_ih[H:2 * H] + b_hh[H:2 * H]),
                         b_hh[2 * H:],
                         b_ih[2 * H:]], 0)          # [4H]

    x0 = np.asarray(embed_table, f32)[0]
    gi0 = w_ih @ x0 + b_ih                          # [3H]
    bc0 = np.concatenate([gi0[:H] + b_hh[:H],
                          -(gi0[H:2 * H] + b_hh[H:2 * H]),
                          b_hh[2 * H:],
                          gi0[2 * H:]], 0)          # [4H]
    W0 = np.concatenate([w_hh[:H], -w_hh[H:2 * H]], 0)  # [2H, H] r,z step-0

    # stationary blocks: wst[kp, ((g*4+q)*4+c)*128 + m] = Wc[g*512+q*128+m,
    #                                                        c*128+kp]
    wst = np.empty((128, 64 * 128), f32)
    for g in range(4):
        for q in range(4):
            for c in range(4):
                blk = ((g * 4 + q) * 4 + c) * 128
                wst[:, blk:blk + 128] = Wc[g * 512 + q * 128:
                                           g * 512 + (q + 1) * 128,
                                           c * 128:(c + 1) * 128].T
    wst0 = np.empty((128, 32 * 128), f32)
    for g in range(2):
        for q in range(4):
            for c in range(4):
                blk = ((g * 4 + q) * 4 + c) * 128
                wst0[:, blk:blk + 128] = W0[g * 512 + q * 128:
                                            g * 512 + (q + 1) * 128,
                                            c * 128:(c + 1) * 128].T

    proj_w = np.asarray(proj_w, f32)                # [V, H]
    pjt = np.empty((128, 4 * VOCAB), f32)
    for c in range(4):
        pjt[:, c * VOCAB:(c + 1) * VOCAB] = proj_w[:, c * 128:(c + 1) * 128].T

    feat = np.asarray(feat, f32)
    common = {
        "wst": wst.astype(f16),
        "wst0": wst0.astype(f16),
        "bt": bc.reshape(1, 2048).astype(f16),
        "bt0": bc0.reshape(1, 2048).astype(f16),
        "ones": np.ones((1, BD), f16),
        "pjt": pjt.astype(f16),
        "pjb": np.asarray(proj_b, f32).reshape(VOCAB, 1),
    }
    maps = []
    for i in range(NCORES):
        fs = feat[i * BD:(i + 1) * BD]              # [BD, H]
        h0g = np.ascontiguousarray(
            fs.T.reshape(4, 128, BD).transpose(1, 0, 2).reshape(128, 128))
        maps.append(dict(common, h0=h0g.astype(f16)))
    return maps


def kernel(feat, embed_table, w_ih, w_hh, b_ih, b_hh, proj_w, proj_b,
           _trace=False):
    nc = _get_nc(STEPS)
    in_maps = _prep_inputs(feat, embed_table, w_ih, w_hh, b_ih, b_hh,
                           proj_w, proj_b)
    res = run_bass_kernel_spmd(nc, in_maps, list(range(NCORES)), trace=_trace)
    out = np.concatenate([res.results[i]["out"] for i in range(NCORES)], 0)
    if _trace:
        kernel.last_exec_time_ns = res.exec_time_ns
        kernel.last_results = res
    return out
